# revision 11
# baseline (speedup 1.0000x reference)
"""Causal self-attention (T=2048, C=1024, H=16) on 8 Trainium2 NeuronCores.

Tensor-parallel over heads: each core owns 2 heads (wqkv row-shard), computes
qkv + attention for its heads, all-gathers the per-head attention outputs, then
computes its 128-column slice of the output projection (proj column-shard).

Layout notes (per core c, heads 2c and 2c+1):
  - xT   [1024, 2048]  x transposed (shared by all cores)
  - wT   [1024, 384]   wqkv rows for (q,k,v) of this core's heads, transposed;
                       q-rows pre-scaled by 1/sqrt(64)=0.125 (exact)
  - qT/kT/vT [128, 2048] in SBUF: rows = 2 heads x 64 dims, cols = tokens
  - scoresT  [s, t] tiles computed directly (no probs transpose needed);
    softmax denominator comes from an appended ones-column on v (row 64 of the
    attnT psum accumulator), so no cross-partition reductions are needed.
  - exp() is safe without max-subtraction: |scores| < 4 for this problem.

All matmul operands use float32r (single-pass PE, 4x the fp32 rate); set
KERNEL_FP32_EXACT=1 to fall back to exact fp32 matmuls.
"""

import os
import numpy as np

import concourse.bass as bass
import concourse.mybir as mybir
import concourse.tile as tile
from concourse import bacc
from concourse import bass_utils

T = 2048
C = 1024
H = 16
D = 64
N_CORES = 8
P = 128
NT = T // P          # 16 token tiles
NG = T // 512        # 4 column chunks of 512
NO = C // P          # 8 contraction subtiles

F32 = mybir.dt.float32
# float32r: single-pass PE matmul (4x faster than float32's hi/lo split).
FAST_MM = os.environ.get("KERNEL_FP32_EXACT", "0") != "1"
MM = mybir.dt.float32r if FAST_MM else mybir.dt.float32


def _build():
    nc = bacc.Bacc("TRN2", target_bir_lowering=False, debug=False,
                   num_devices=N_CORES)

    xT = nc.dram_tensor("xT", [C, T], MM, kind="ExternalInput").ap()
    wT = nc.dram_tensor("wT", [C, 3 * P], MM, kind="ExternalInput").ap()
    bqkv = nc.dram_tensor("bqkv", [P, 3], F32, kind="ExternalInput").ap()
    pwT = nc.dram_tensor("pwT", [C, P], MM, kind="ExternalInput").ap()
    pb = nc.dram_tensor("pb", [P, 1], F32, kind="ExternalInput").ap()
    mask01 = nc.dram_tensor("mask01", [P, P], MM, kind="ExternalInput").ap()
    onesd = nc.dram_tensor("onesd", [P, D], MM, kind="ExternalInput").ap()
    ident = nc.dram_tensor("ident", [P, P], MM, kind="ExternalInput").ap()
    outT = nc.dram_tensor("outT", [P, T], F32, kind="ExternalOutput").ap()

    xT3 = xT.rearrange("(o p) t -> p o t", p=P)      # [128, 8, 2048]
    wT3 = wT.rearrange("(o p) j -> p o j", p=P)      # [128, 8, 384]
    pwT3 = pwT.rearrange("(o p) j -> p o j", p=P)    # [128, 8, 128]

    with tile.TileContext(nc) as tc:
        with (
            tc.tile_pool(name="const", bufs=1) as constp,
            tc.tile_pool(name="big", bufs=1) as bigp,
            tc.tile_pool(name="work", bufs=3) as workp,
            tc.tile_pool(name="outp", bufs=3) as outp,
            tc.tile_pool(name="dram", bufs=1, space="DRAM") as dram,
        ):
            # ---- constants ----
            mask_sb = constp.tile([P, P], MM, name="mask")
            nc.sync.dma_start(mask_sb[:], mask01)
            id_sb = constp.tile([P, P], MM, name="ident")
            nc.sync.dma_start(id_sb[:], ident)
            wT_sb = constp.tile([P, NO, 3 * P], MM, name="wT")
            nc.sync.dma_start(wT_sb[:], wT3)
            bq_sb = constp.tile([P, 3], F32, name="bqkv")
            nc.sync.dma_start(bq_sb[:], bqkv)
            pwT_sb = constp.tile([P, NO, P], MM, name="pwT")
            nc.sync.dma_start(pwT_sb[:], pwT3)
            pb_sb = constp.tile([P, 1], F32, name="pb")
            nc.sync.dma_start(pb_sb[:], pb)
            ones_sb = constp.tile([1, D], MM, name="ones")
            nc.sync.dma_start(ones_sb[:], onesd[0:1, :])
            onesv_sb = constp.tile([P, NT], MM, name="onesv")
            nc.sync.dma_start(onesv_sb[:], onesd[:, 0:NT])

            x_sb = bigp.tile([P, NO, T], MM, name="x")
            for o in range(NO):
                nc.sync.dma_start(x_sb[:, o, :], xT3[:, o, :])

            # ---- fused QKV projection (transposed outputs) ----
            qkvT = [bigp.tile([P, T], MM, name=n) for n in ("qT", "kT", "vT")]
            ps_qkv = tc.alloc_tile_pool(name="ps_qkv", bufs=2, space="PSUM")
            ps_vtr = tc.alloc_tile_pool(name="ps_vtr", bufs=2, space="PSUM")
            for j in range(3):
                for g in range(NG):
                    ps = ps_qkv.tile([P, 512], F32, name="qkv")
                    for o in range(NO):
                        nc.tensor.matmul(
                            ps[:],
                            lhsT=wT_sb[:, o, j * P:(j + 1) * P],
                            rhs=x_sb[:, o, g * 512:(g + 1) * 512],
                            start=(o == 0), stop=(o == NO - 1),
                        )
                    # add bias while copying psum -> sbuf
                    nc.scalar.activation(
                        qkvT[j][:, g * 512:(g + 1) * 512], ps[:],
                        mybir.ActivationFunctionType.Identity,
                        bias=bq_sb[:, j:j + 1],
                    )
            qT_sb, kT_sb, vT_sb = qkvT

            # ---- v: transpose to [s, d] tiles, with ones column per head ----
            # layout: v_sb[:, st, 0:64]=head0 dims, [64]=1.0, [65:129]=head1, [129]=1.0
            v_sb = bigp.tile([P, NT, 130], MM, name="v")
            nc.vector.tensor_copy(out=v_sb[:, :, D:D + 1], in_=onesv_sb[:, :, None])
            nc.vector.tensor_copy(out=v_sb[:, :, 2 * D + 1:2 * D + 2],
                                  in_=onesv_sb[:, :, None])
            for st in range(NT):
                pst = ps_vtr.tile([P, P], MM, name="vtr")
                nc.tensor.transpose(pst[:], vT_sb[:, st * P:(st + 1) * P],
                                    id_sb[:])
                nc.vector.tensor_copy(out=v_sb[:, st, 0:D], in_=pst[:, 0:D])
                nc.vector.tensor_copy(out=v_sb[:, st, D + 1:2 * D + 1],
                                      in_=pst[:, D:2 * D])

            ps_vtr.release()
            ps_qkv.release()

            # ---- attention (both heads), all-gather input staging ----
            ag_in = dram.tile([P, T], MM)
            ag_out = dram.tile([C, T], MM, addr_space="Shared")

            ps_sc = tc.alloc_tile_pool(name="ps_sc", bufs=3, space="PSUM")
            ps_at = tc.alloc_tile_pool(name="ps_at", bufs=2, space="PSUM")
            ps_rb = tc.alloc_tile_pool(name="ps_rb", bufs=2, space="PSUM")
            for h in range(2):
                qTh = qT_sb[h * D:(h + 1) * D, :]
                kTh = kT_sb[h * D:(h + 1) * D, :]
                for g in range(NG):
                    at = ps_at.tile([P, 512], F32, name="at")
                    for j in range(4 * g + 4):
                        t0 = 512 * g if j < 4 * g else P * j
                        w_ = 512 * (g + 1) - t0
                        sc = ps_sc.tile([P, 512], F32, name="sc")
                        nc.tensor.matmul(
                            sc[:, :w_],
                            lhsT=kTh[:, j * P:(j + 1) * P],
                            rhs=qTh[:, t0:t0 + w_],
                            start=True, stop=True,
                        )
                        e_sb = workp.tile([P, 512], MM, name="e")
                        nc.scalar.activation(e_sb[:, :w_], sc[:, :w_],
                                             mybir.ActivationFunctionType.Exp)
                        if j >= 4 * g:
                            # zero the strictly-upper (t<s) part of the diag block
                            nc.vector.tensor_mul(out=e_sb[:, 0:P],
                                                 in0=e_sb[:, 0:P],
                                                 in1=mask_sb[:])
                        nc.tensor.matmul(
                            at[:D + 1, t0 - 512 * g:512],
                            lhsT=v_sb[:, j, h * (D + 1):(h + 1) * (D + 1)],
                            rhs=e_sb[:, :w_],
                            start=(j == 0), stop=(j == 4 * g + 3),
                        )
                    # rows 0..63 = unnormalized attnT, row 64 = softmax denom
                    rs = workp.tile([1, 512], F32, name="rs")
                    nc.vector.tensor_copy(out=rs[:], in_=at[D:D + 1, :])
                    ri = workp.tile([1, 512], MM, name="ri")
                    with nc.allow_low_precision(
                            reason="float32r rounding only; feeds f32r matmul"):
                        nc.vector.reciprocal(ri[:], rs[:])
                    rb = ps_rb.tile([D, 512], F32, name="rb")
                    nc.tensor.matmul(rb[:], lhsT=ones_sb[:], rhs=ri[:],
                                     start=True, stop=True)
                    rr = workp.tile([D, 512], F32, name="rr")
                    nc.vector.tensor_copy(out=rr[:], in_=rb[:])
                    ao = outp.tile([D, 512], MM, name="ao")
                    nc.vector.tensor_mul(out=ao[:], in0=at[0:D, :], in1=rr[:])
                    nc.sync.dma_start(
                        ag_in[h * D:(h + 1) * D, g * 512:(g + 1) * 512], ao[:])

            ps_rb.release()
            ps_at.release()
            ps_sc.release()

            ps_proj = tc.alloc_tile_pool(name="ps_proj", bufs=2, space="PSUM")
            nc.gpsimd.collective_compute(
                "AllGather",
                mybir.AluOpType.bypass,
                replica_groups=[list(range(N_CORES))],
                ins=[ag_in.opt()],
                outs=[ag_out.opt()],
            )

            # ---- output projection (128-column slice of out, transposed) ----
            ag3 = ag_out[:].rearrange("(o p) t -> p o t", p=P)
            for g in range(NG):
                ps = ps_proj.tile([P, 512], F32, name="proj")
                for o in range(NO):
                    r_sb = workp.tile([P, 512], MM, name="agr")
                    nc.sync.dma_start(r_sb[:], ag3[:, o, g * 512:(g + 1) * 512])
                    nc.tensor.matmul(ps[:], lhsT=pwT_sb[:, o, :],
                                     rhs=r_sb[:],
                                     start=(o == 0), stop=(o == NO - 1))
                ob = outp.tile([P, 512], F32, name="ob")
                nc.scalar.activation(ob[:], ps[:],
                                     mybir.ActivationFunctionType.Identity,
                                     bias=pb_sb[:])
                nc.sync.dma_start(outT[:, g * 512:(g + 1) * 512], ob[:])
            ps_proj.release()

    nc.compile()
    return nc


_NC = None
LAST_RESULT = None


def _get_nc():
    global _NC
    if _NC is None:
        _NC = _build()
    return _NC


def _prep_inputs(x, wqkv_w, wqkv_b, proj_w, proj_b):
    x = np.asarray(x, np.float32)
    wqkv_w = np.asarray(wqkv_w, np.float32)
    wqkv_b = np.asarray(wqkv_b, np.float32)
    proj_w = np.asarray(proj_w, np.float32)
    proj_b = np.asarray(proj_b, np.float32)

    scale = np.float32(1.0 / np.sqrt(D))  # 0.125 exactly
    xT = np.ascontiguousarray(x.T)
    mask = np.triu(np.ones((P, P), np.float32))  # mask[s,t] = 1 if t>=s
    eye = np.eye(P, dtype=np.float32)

    in_maps = []
    for c in range(N_CORES):
        qs = slice(P * c, P * (c + 1))
        ks = slice(C + P * c, C + P * (c + 1))
        vs = slice(2 * C + P * c, 2 * C + P * (c + 1))
        w_c = np.concatenate(
            [wqkv_w[qs] * scale, wqkv_w[ks], wqkv_w[vs]], axis=0)  # [384, 1024]
        b_c = np.concatenate(
            [wqkv_b[qs] * scale, wqkv_b[ks], wqkv_b[vs]])          # [384]
        in_maps.append({
            "xT": xT,
            "wT": np.ascontiguousarray(w_c.T),
            "bqkv": np.ascontiguousarray(b_c.reshape(3, P).T),
            "pwT": np.ascontiguousarray(proj_w[qs].T),
            "pb": np.ascontiguousarray(proj_b[qs].reshape(P, 1)),
            "mask01": mask,
            "onesd": np.ones((P, D), np.float32),
            "ident": eye,
        })
    return in_maps


def kernel(x, wqkv_w, wqkv_b, proj_w, proj_b):
    global LAST_RESULT
    nc = _get_nc()
    in_maps = _prep_inputs(x, wqkv_w, wqkv_b, proj_w, proj_b)
    res = bass_utils.run_bass_kernel_spmd(nc, in_maps,
                                          core_ids=list(range(N_CORES)))
    LAST_RESULT = res
    full_outT = np.concatenate(
        [res.results[c]["outT"] for c in range(N_CORES)], axis=0)  # [1024, 2048]
    return np.ascontiguousarray(full_outT.T).astype(np.float32)


# revision 13
# speedup vs baseline: 1.2251x; 1.2251x over previous
"""Causal self-attention (T=2048, C=1024, H=16) on 8 Trainium2 NeuronCores.

Tensor-parallel over heads: each core owns 2 heads (wqkv row-shard), computes
qkv + attention for its heads, all-gathers the per-head attention outputs, then
computes its 128-column slice of the output projection (proj column-shard).

Layout notes (per core c, heads 2c and 2c+1):
  - xT   [1024, 2048]  x transposed (shared by all cores)
  - wT   [1024, 384]   wqkv rows for (q,k,v) of this core's heads, transposed;
                       q-rows pre-scaled by 1/sqrt(64)=0.125 (exact)
  - qT/kT/vT [128, 2048] in SBUF: rows = 2 heads x 64 dims, cols = tokens
  - scoresT  [s, t] tiles computed directly (no probs transpose needed);
    softmax denominator comes from an appended ones-column on v (row 64 of the
    attnT psum accumulator), so no cross-partition reductions are needed.
  - exp() is safe without max-subtraction: |scores| < 4 for this problem.
  - the AllGather is split into 4 per-t-chunk collectives so gather overlaps
    attention compute and the projection overlaps the later gathers.

All matmul operands use float32r (single-pass PE, 4x the fp32 rate); set
KERNEL_FP32_EXACT=1 to fall back to exact fp32 matmuls.
"""

import os
import numpy as np

import concourse.bass as bass
import concourse.mybir as mybir
import concourse.tile as tile
from concourse import bacc
from concourse import bass_utils

T = 2048
C = 1024
H = 16
D = 64
N_CORES = 8
P = 128
NT = T // P          # 16 token tiles
NG = T // 512        # 4 column chunks of 512
NO = C // P          # 8 contraction subtiles

F32 = mybir.dt.float32
# float32r: single-pass PE matmul (4x faster than float32's hi/lo split).
FAST_MM = os.environ.get("KERNEL_FP32_EXACT", "0") != "1"
MM = mybir.dt.float32r if FAST_MM else mybir.dt.float32


def _build():
    nc = bacc.Bacc("TRN2", target_bir_lowering=False, debug=False,
                   num_devices=N_CORES)

    xT = nc.dram_tensor("xT", [C, T], MM, kind="ExternalInput").ap()
    wT = nc.dram_tensor("wT", [C, 3 * P], MM, kind="ExternalInput").ap()
    bqkv = nc.dram_tensor("bqkv", [P, 3], F32, kind="ExternalInput").ap()
    pwT = nc.dram_tensor("pwT", [C, P], MM, kind="ExternalInput").ap()
    pb = nc.dram_tensor("pb", [P, 1], F32, kind="ExternalInput").ap()
    mask01 = nc.dram_tensor("mask01", [P, P], MM, kind="ExternalInput").ap()
    onesd = nc.dram_tensor("onesd", [P, D], MM, kind="ExternalInput").ap()
    ident = nc.dram_tensor("ident", [P, P], MM, kind="ExternalInput").ap()
    outT = nc.dram_tensor("outT", [P, T], F32, kind="ExternalOutput").ap()

    xT3 = xT.rearrange("(o p) t -> p o t", p=P)      # [128, 8, 2048]
    wT3 = wT.rearrange("(o p) j -> p o j", p=P)      # [128, 8, 384]
    pwT3 = pwT.rearrange("(o p) j -> p o j", p=P)    # [128, 8, 128]

    with tile.TileContext(nc) as tc:
        with (
            tc.tile_pool(name="const", bufs=1) as constp,
            tc.tile_pool(name="big", bufs=1) as bigp,
            tc.tile_pool(name="work", bufs=4) as workp,
            tc.tile_pool(name="projw", bufs=2) as projwp,
            tc.tile_pool(name="outp", bufs=3) as outp,
            tc.tile_pool(name="dram", bufs=1, space="DRAM") as dram,
        ):
            # ---- constants ----
            mask_sb = constp.tile([P, P], MM, name="mask")
            nc.sync.dma_start(mask_sb[:], mask01)
            id_sb = constp.tile([P, P], MM, name="ident")
            nc.sync.dma_start(id_sb[:], ident)
            wT_sb = constp.tile([P, NO, 3 * P], MM, name="wT")
            nc.sync.dma_start(wT_sb[:], wT3)
            bq_sb = constp.tile([P, 3], F32, name="bqkv")
            nc.sync.dma_start(bq_sb[:], bqkv)
            pwT_sb = constp.tile([P, NO, P], MM, name="pwT")
            nc.sync.dma_start(pwT_sb[:], pwT3)
            pb_sb = constp.tile([P, 1], F32, name="pb")
            nc.sync.dma_start(pb_sb[:], pb)
            ones_sb = constp.tile([1, D], F32, name="ones")
            nc.vector.memset(ones_sb[:], 1.0)
            onesv_sb = constp.tile([P, NT], MM, name="onesv")
            nc.sync.dma_start(onesv_sb[:], onesd[:, 0:NT])

            # x input: one DMA per 128-row subtile, pipelined into the QKV loop
            x_sb = bigp.tile([P, NO, T], MM, name="x")
            for o in range(NO):
                nc.sync.dma_start(x_sb[:, o, :], xT3[:, o, :])

            # ---- fused QKV projection (transposed outputs) ----
            # o-outer so matmuls start as soon as x subtile 0 has landed.
            qkvT = [bigp.tile([P, T], MM, name=n) for n in ("qT", "kT", "vT")]
            ps_qkv = tc.alloc_tile_pool(name="ps_qkv", bufs=1, space="PSUM")
            ps_vtr = tc.alloc_tile_pool(name="ps_vtr", bufs=2, space="PSUM")
            for j in range(3):
                pss = [ps_qkv.tile([P, 512], F32, name=f"qkv{g}")
                       for g in range(NG)]
                for o in range(NO):
                    for g in range(NG):
                        nc.tensor.matmul(
                            pss[g][:],
                            lhsT=wT_sb[:, o, j * P:(j + 1) * P],
                            rhs=x_sb[:, o, g * 512:(g + 1) * 512],
                            start=(o == 0), stop=(o == NO - 1),
                        )
                for g in range(NG):
                    # add bias while copying psum -> sbuf
                    nc.scalar.activation(
                        qkvT[j][:, g * 512:(g + 1) * 512], pss[g][:],
                        mybir.ActivationFunctionType.Identity,
                        bias=bq_sb[:, j:j + 1],
                    )
            qT_sb, kT_sb, vT_sb = qkvT

            # ---- v: transpose to [s, d] tiles, with ones column per head ----
            # layout: v_sb[:, st, 0:64]=head0 dims, [64]=1.0, [65:129]=head1, [129]=1.0
            v_sb = bigp.tile([P, NT, 130], MM, name="v")
            nc.vector.tensor_copy(out=v_sb[:, :, D:D + 1], in_=onesv_sb[:, :, None])
            nc.vector.tensor_copy(out=v_sb[:, :, 2 * D + 1:2 * D + 2],
                                  in_=onesv_sb[:, :, None])
            for st in range(NT):
                pst = ps_vtr.tile([P, P], MM, name="vtr")
                nc.tensor.transpose(pst[:], vT_sb[:, st * P:(st + 1) * P],
                                    id_sb[:])
                nc.vector.tensor_copy(out=v_sb[:, st, 0:D], in_=pst[:, 0:D])
                nc.vector.tensor_copy(out=v_sb[:, st, D + 1:2 * D + 1],
                                      in_=pst[:, D:2 * D])

            ps_vtr.release()
            ps_qkv.release()

            # ---- attention; per-t-chunk staged AllGather + projection ----
            ag_ins = [dram.tile([P, 512], MM, name=f"agi{g}") for g in range(NG)]
            ag_outs = [dram.tile([C, 512], MM, addr_space="Shared",
                                 name=f"ago{g}") for g in range(NG)]

            ps_sc = tc.alloc_tile_pool(name="ps_sc", bufs=4, space="PSUM")
            ps_at = tc.alloc_tile_pool(name="ps_at", bufs=2, space="PSUM")
            ps_rb = tc.alloc_tile_pool(name="ps_rb", bufs=2, space="PSUM")

            for g in range(NG):
                ats = [ps_at.tile([P, 512], F32, name="at") for _ in range(2)]
                # both heads interleaved: two independent chains keep the PE
                # busy while the other head's exp() runs on the scalar engine
                for j in range(4 * g + 4):
                    t0 = 512 * g if j < 4 * g else P * j
                    w_ = 512 * (g + 1) - t0
                    for h in range(2):
                        qTh = qT_sb[h * D:(h + 1) * D, :]
                        kTh = kT_sb[h * D:(h + 1) * D, :]
                        sc = ps_sc.tile([P, 512], F32, name="sc")
                        nc.tensor.matmul(
                            sc[:, :w_],
                            lhsT=kTh[:, j * P:(j + 1) * P],
                            rhs=qTh[:, t0:t0 + w_],
                            start=True, stop=True,
                        )
                        e_sb = workp.tile([P, 512], MM, name="e")
                        nc.scalar.activation(e_sb[:, :w_], sc[:, :w_],
                                             mybir.ActivationFunctionType.Exp)
                        if j >= 4 * g:
                            # zero the strictly-upper (t<s) part of the diag block
                            nc.vector.tensor_mul(out=e_sb[:, 0:P],
                                                 in0=e_sb[:, 0:P],
                                                 in1=mask_sb[:])
                        nc.tensor.matmul(
                            ats[h][:D + 1, t0 - 512 * g:512],
                            lhsT=v_sb[:, j, h * (D + 1):(h + 1) * (D + 1)],
                            rhs=e_sb[:, :w_],
                            start=(j == 0), stop=(j == 4 * g + 3),
                        )
                for h in range(2):
                    at = ats[h]
                    # rows 0..63 = unnormalized attnT, row 64 = softmax denom
                    rs = workp.tile([1, 512], F32, name="rs")
                    nc.vector.tensor_copy(out=rs[:], in_=at[D:D + 1, :])
                    rb = ps_rb.tile([D, 512], F32, name="rb")
                    nc.tensor.matmul(rb[:], lhsT=ones_sb[:], rhs=rs[:],
                                     start=True, stop=True)
                    rr = workp.tile([D, 512], F32, name="rr")
                    nc.vector.reciprocal(rr[:], rb[:])
                    ao = outp.tile([D, 512], MM, name="ao")
                    nc.vector.tensor_mul(out=ao[:], in0=at[0:D, :], in1=rr[:])
                    nc.sync.dma_start(ag_ins[g][h * D:(h + 1) * D, :], ao[:])
                nc.gpsimd.collective_compute(
                    "AllGather",
                    mybir.AluOpType.bypass,
                    replica_groups=[list(range(N_CORES))],
                    ins=[ag_ins[g].opt()],
                    outs=[ag_outs[g].opt()],
                )

            ps_rb.release()
            ps_at.release()
            ps_sc.release()

            # ---- output projection (128-column slice of out, transposed) ----
            ps_proj = tc.alloc_tile_pool(name="ps_proj", bufs=2, space="PSUM")
            for g in range(NG):
                ag3 = ag_outs[g][:].rearrange("(o p) t -> p o t", p=P)
                r_sb = projwp.tile([P, NO, 512], MM, name="agr")
                nc.sync.dma_start(r_sb[:], ag3[:])
                ps = ps_proj.tile([P, 512], F32, name="proj")
                for o in range(NO):
                    nc.tensor.matmul(ps[:], lhsT=pwT_sb[:, o, :],
                                     rhs=r_sb[:, o, :],
                                     start=(o == 0), stop=(o == NO - 1))
                ob = outp.tile([P, 512], F32, name="ob")
                nc.scalar.activation(ob[:], ps[:],
                                     mybir.ActivationFunctionType.Identity,
                                     bias=pb_sb[:])
                nc.sync.dma_start(outT[:, g * 512:(g + 1) * 512], ob[:])
            ps_proj.release()

    nc.compile()
    return nc


_NC = None
LAST_RESULT = None


def _get_nc():
    global _NC
    if _NC is None:
        _NC = _build()
    return _NC


def _prep_inputs(x, wqkv_w, wqkv_b, proj_w, proj_b):
    x = np.asarray(x, np.float32)
    wqkv_w = np.asarray(wqkv_w, np.float32)
    wqkv_b = np.asarray(wqkv_b, np.float32)
    proj_w = np.asarray(proj_w, np.float32)
    proj_b = np.asarray(proj_b, np.float32)

    scale = np.float32(1.0 / np.sqrt(D))  # 0.125 exactly
    xT = np.ascontiguousarray(x.T)
    mask = np.triu(np.ones((P, P), np.float32))  # mask[s,t] = 1 if t>=s
    eye = np.eye(P, dtype=np.float32)

    in_maps = []
    for c in range(N_CORES):
        qs = slice(P * c, P * (c + 1))
        ks = slice(C + P * c, C + P * (c + 1))
        vs = slice(2 * C + P * c, 2 * C + P * (c + 1))
        w_c = np.concatenate(
            [wqkv_w[qs] * scale, wqkv_w[ks], wqkv_w[vs]], axis=0)  # [384, 1024]
        b_c = np.concatenate(
            [wqkv_b[qs] * scale, wqkv_b[ks], wqkv_b[vs]])          # [384]
        in_maps.append({
            "xT": xT,
            "wT": np.ascontiguousarray(w_c.T),
            "bqkv": np.ascontiguousarray(b_c.reshape(3, P).T),
            "pwT": np.ascontiguousarray(proj_w[qs].T),
            "pb": np.ascontiguousarray(proj_b[qs].reshape(P, 1)),
            "mask01": mask,
            "onesd": np.ones((P, D), np.float32),
            "ident": eye,
        })
    return in_maps


def kernel(x, wqkv_w, wqkv_b, proj_w, proj_b):
    global LAST_RESULT
    nc = _get_nc()
    in_maps = _prep_inputs(x, wqkv_w, wqkv_b, proj_w, proj_b)
    res = bass_utils.run_bass_kernel_spmd(nc, in_maps,
                                          core_ids=list(range(N_CORES)))
    LAST_RESULT = res
    full_outT = np.concatenate(
        [res.results[c]["outT"] for c in range(N_CORES)], axis=0)  # [1024, 2048]
    return np.ascontiguousarray(full_outT.T).astype(np.float32)
